# revision 26
# baseline (speedup 1.0000x reference)
"""Trainium2 Bass kernel for a 12-layer BERT generator model.

Model (see problem reference):
  B=8, S=512, H=768, L=12, NH=12 (DH=64), FF=3072, V=21128
  - embedding gather + pos/type embeddings + LN
  - L x { QA-causal masked multi-head attention + LN, exact-GELU FFN + LN }
  - vocab projection [S, V]

Sharding: data-parallel, one sample per NeuronCore (B == n_cores == 8).
Each core runs an identical program; in_maps carry the per-core sample ids
plus (replicated) weights.

Per-core layout conventions:
  - activations are FEATURE-major in SBUF: hT[p, kf, s] = h[s, kf*128+p]
    (shape [128, H//128, S]) so that every dense layer is
        out[f_out, s] = sum_f W[f, f_out] * hT[f, s]
    i.e. matmul(lhsT=W-slice [128, 128], rhs=hT-slice [128, S]) with no
    transposes between layers.
  - k and v are ALSO produced token-major (v_tok[s, f]) "for free" by
    swapping matmul operands; attention probabilities are transposed with
    the PE transpose instruction.
  - heavy projection matmuls (QKV/O/FFN/vocab, attention ctx) run in bf16
    (weights host-cast to bf16, activations cast once per phase) with fp32
    PSUM accumulation; the residual stream, LayerNorms, attention scores
    (q.k), softmax, and all biases stay in full fp32.
"""

import sys

sys.path.insert(0, "/opt/trn_rl_repo")

import numpy as np

import concourse.bass as bass
import concourse.mybir as mybir
import concourse.tile as tile
from concourse import bacc
from concourse.bass import IndirectOffsetOnAxis
from concourse.masks import make_identity

P = 128
PAD_ID, SEP_ID = 0, 102
EPS = 1e-12
NEG = -1e9

F32 = mybir.dt.float32
F32R = mybir.dt.float32r
BF16 = mybir.dt.bfloat16
I32 = mybir.dt.int32

FULL_CFG = dict(S=512, H=768, NH=12, L=12, FF=3072, V=21128)
N_CORES = 8


def _r(ap):
    """Reinterpret an fp32 AP as float32r for fast matmul."""
    return ap.bitcast(F32R)


def _nslices(total, step=512):
    out = []
    s = 0
    while s < total:
        out.append((s, min(step, total - s)))
        s += step
    return out


def build_nc(cfg, debug=False):
    S, H, NH, L, FF, V = (
        cfg["S"], cfg["H"], cfg["NH"], cfg["L"], cfg["FF"], cfg["V"],
    )
    DH = 64
    assert H % P == 0 and S % P == 0 and FF % P == 0
    assert H // NH == DH
    KF = H // P          # feature subtiles (6)
    SC = S // P          # token chunks (4)
    KFF = FF // P        # ffn subtiles (24)
    HPG = P // DH        # heads per 128-partition group (2)

    nc = bacc.Bacc("TRN2", target_bir_lowering=False, debug=debug)

    # ---- I/O ------------------------------------------------------------
    ids_d = nc.declare_dram_parameter("input_ids", [S], I32, False)
    wemb_d = nc.declare_dram_parameter("word_emb", [V, H], F32, False)
    # pos_emb + type_emb[0] folded on the host
    pt_d = nc.declare_dram_parameter("pos_type_emb", [S, H], F32, False)
    embg_d = nc.declare_dram_parameter("emb_ln_g", [H], F32, False)
    embb_d = nc.declare_dram_parameter("emb_ln_b", [H], F32, False)
    aw_d = nc.declare_dram_parameter("attn_w", [L, 4, H, H], BF16, False)
    ab_d = nc.declare_dram_parameter("attn_b", [L, 4, H], F32, False)
    l1g_d = nc.declare_dram_parameter("ln1_g", [L, H], F32, False)
    l1b_d = nc.declare_dram_parameter("ln1_b", [L, H], F32, False)
    w1_d = nc.declare_dram_parameter("ffn_w1", [L, H, FF], BF16, False)
    b1_d = nc.declare_dram_parameter("ffn_b1", [L, FF], F32, False)
    w2_d = nc.declare_dram_parameter("ffn_w2", [L, FF, H], BF16, False)
    b2_d = nc.declare_dram_parameter("ffn_b2", [L, H], F32, False)
    l2g_d = nc.declare_dram_parameter("ln2_g", [L, H], F32, False)
    l2b_d = nc.declare_dram_parameter("ln2_b", [L, H], F32, False)
    clsw_d = nc.declare_dram_parameter("cls_w", [H, V], BF16, False)
    clsb_d = nc.declare_dram_parameter("cls_b", [V], F32, False)
    out_d = nc.declare_dram_parameter("out", [S, V], F32, True)

    with tile.TileContext(nc) as tc:
        _build_body(
            nc, tc,
            dict(S=S, H=H, NH=NH, L=L, FF=FF, V=V, DH=DH, KF=KF, SC=SC,
                 KFF=KFF, HPG=HPG),
            dict(ids=ids_d, wemb=wemb_d, pt=pt_d, embg=embg_d, embb=embb_d,
                 aw=aw_d, ab=ab_d, l1g=l1g_d, l1b=l1b_d, w1=w1_d, b1=b1_d,
                 w2=w2_d, b2=b2_d, l2g=l2g_d, l2b=l2b_d, clsw=clsw_d,
                 clsb=clsb_d, out=out_d),
        )
    nc.compile()
    return nc


def _build_body(nc, tc, c, d):
    S, H, NH, L, FF, V = c["S"], c["H"], c["NH"], c["L"], c["FF"], c["V"]
    DH, KF, SC, KFF, HPG = c["DH"], c["KF"], c["SC"], c["KFF"], c["HPG"]
    AL = mybir.AluOpType
    AF = mybir.ActivationFunctionType
    AX = mybir.AxisListType

    import contextlib

    ctx = contextlib.ExitStack()
    with ctx:
        const = ctx.enter_context(tc.tile_pool(name="const", bufs=1))
        persist = ctx.enter_context(tc.tile_pool(name="persist", bufs=1))
        wpool = ctx.enter_context(tc.tile_pool(name="wpool", bufs=4))
        w2pool = ctx.enter_context(tc.tile_pool(name="w2pool", bufs=4))
        bpool = ctx.enter_context(tc.tile_pool(name="bpool", bufs=2))
        spool = ctx.enter_context(tc.tile_pool(name="spool", bufs=3))
        ppool = ctx.enter_context(tc.tile_pool(name="ppool", bufs=2))
        opool = ctx.enter_context(tc.tile_pool(name="opool", bufs=3))
        psum = ctx.enter_context(tc.tile_pool(name="psum", bufs=8, space="PSUM"))

        # ---- constants --------------------------------------------------
        ident = const.tile([P, P], F32, name="ident")
        make_identity(nc, ident)
        ident_bf = const.tile([P, P], BF16, name="ident_bf")
        make_identity(nc, ident_bf)
        ones_col = const.tile([P, 1], F32, name="ones_col")   # lhsT for column sums
        nc.gpsimd.memset(ones_col[:], 1.0)
        ones_col_bf = const.tile([P, 1], BF16, name="ones_col_bf")
        nc.gpsimd.memset(ones_col_bf[:], 1.0)
        ones_row = const.tile([1, P], F32, name="ones_row")   # lhsT for broadcasts
        nc.gpsimd.memset(ones_row[:], 1.0)
        eps_col = const.tile([P, 1], F32, name="eps_col")
        nc.gpsimd.memset(eps_col[:], EPS)

        # ---- persistent activations ------------------------------------
        hT = persist.tile([P, KF, S], F32, name="hT")
        hT_bf = persist.tile([P, KF, S], BF16, name="hT_bf")
        qT = persist.tile([P, KF, S], BF16, name="qT")
        kT = persist.tile([P, KF, S], BF16, name="kT")
        v_tok = persist.tile([P, SC, H], BF16, name="v_tok")
        ctxT = persist.tile([P, KF, S], BF16, name="ctxT")
        attn_bias = persist.tile([P, SC, S], BF16, name="attn_bias")

        # =================================================================
        # Mask / additive attention bias from input_ids
        # =================================================================
        ids_row_i = spool.tile([1, S], I32, name="ids_row_i")
        nc.sync.dma_start(out=ids_row_i[:], in_=d["ids"][None, :])
        ids_row = const.tile([1, S], F32, name="ids_row")
        nc.vector.tensor_copy(out=ids_row[:], in_=ids_row_i[:])

        ids_p_i = spool.tile([P, SC], I32, name="ids_p_i")
        nc.sync.dma_start(
            out=ids_p_i[:], in_=d["ids"].rearrange("(c p) -> p c", p=P)
        )
        ids_p = const.tile([P, SC], F32, name="ids_p")
        nc.vector.tensor_copy(out=ids_p[:], in_=ids_p_i[:])

        iota_j_i = spool.tile([1, S], I32, name="iota_j_i")
        nc.gpsimd.iota(iota_j_i[:], pattern=[[1, S]], base=0, channel_multiplier=0)
        iota_j = const.tile([1, S], F32, name="iota_j")
        nc.vector.tensor_copy(out=iota_j[:], in_=iota_j_i[:])

        iota_i_i = spool.tile([P, SC], I32, name="iota_i_i")
        nc.gpsimd.iota(iota_i_i[:], pattern=[[P, SC]], base=0, channel_multiplier=1)
        iota_i = const.tile([P, SC], F32, name="iota_i")
        nc.vector.tensor_copy(out=iota_i[:], in_=iota_i_i[:])

        pad_j = const.tile([1, S], F32, name="pad_j")
        nc.vector.tensor_scalar(pad_j[:], ids_row[:], float(PAD_ID), None, AL.not_equal)
        pad_i = const.tile([P, SC], F32, name="pad_i")
        nc.vector.tensor_scalar(pad_i[:], ids_p[:], float(PAD_ID), None, AL.not_equal)

        # first-SEP position -> qlen = pos + 1
        sep = spool.tile([1, S], F32, name="sep", tag="lrow", bufs=4)
        nc.vector.tensor_scalar(sep[:], ids_row[:], float(SEP_ID), None, AL.is_equal)
        tsel = spool.tile([1, S], F32, name="tsel", tag="lrow", bufs=4)
        nc.vector.tensor_scalar(tsel[:], iota_j[:], float(S), None, AL.subtract)
        nc.vector.tensor_tensor(tsel[:], tsel[:], sep[:], AL.mult)
        nc.vector.tensor_scalar(tsel[:], tsel[:], float(S), None, AL.add)
        qlen = const.tile([1, 1], F32, name="qlen")
        nc.vector.tensor_reduce(qlen[:], tsel[:], axis=AX.X, op=AL.min)
        nc.vector.tensor_scalar(qlen[:], qlen[:], 1.0, None, AL.add)

        # broadcast qlen to all partitions (K=1 matmul)
        ps_q = psum.tile([P, 512], F32, name="ps", tag="ps")
        nc.tensor.matmul(ps_q[:, :1], lhsT=ones_row[:], rhs=qlen[:], start=True, stop=True)
        qlen_b = const.tile([P, 1], F32, name="qlen_b")
        nc.vector.tensor_copy(out=qlen_b[:], in_=ps_q[:, :1])

        a_i = const.tile([P, SC], F32, name="a_i")
        nc.vector.tensor_scalar(a_i[:], iota_i[:], qlen_b[:, :1], None, AL.is_ge)
        a_j = spool.tile([1, S], F32, name="a_j", tag="lrow", bufs=4)
        nc.vector.tensor_scalar(a_j[:], iota_j[:], qlen[:, :1], None, AL.is_ge)

        # broadcast a_j and pad_j across partitions
        ps_a = psum.tile([P, 512], F32, name="ps", tag="ps")
        a_jb = const.tile([P, S], F32, name="a_jb")
        for s0, sl in _nslices(S):
            nc.tensor.matmul(ps_a[:, :sl], lhsT=ones_row[:], rhs=a_j[:, s0:s0 + sl],
                             start=True, stop=True)
            nc.vector.tensor_copy(out=a_jb[:, s0:s0 + sl], in_=ps_a[:, :sl])
        ps_p = psum.tile([P, 512], F32, name="ps", tag="ps")
        pad_jb = const.tile([P, S], F32, name="pad_jb")
        for s0, sl in _nslices(S):
            nc.tensor.matmul(ps_p[:, :sl], lhsT=ones_row[:], rhs=pad_j[:, s0:s0 + sl],
                             start=True, stop=True)
            nc.vector.tensor_copy(out=pad_jb[:, s0:s0 + sl], in_=ps_p[:, :sl])

        for sc in range(SC):
            # U_c[p, j] = 1.0 if j > sc*128 + p else 0.0
            u_c = spool.tile([P, S], F32, name="u_c", tag="mask_s", bufs=2)
            nc.gpsimd.memset(u_c[:], 1.0)
            nc.gpsimd.affine_select(
                out=u_c[:], in_=u_c[:], compare_op=AL.is_gt, fill=0.0,
                base=-(sc * P), channel_multiplier=-1, pattern=[[1, S]],
            )
            t1 = spool.tile([P, S], F32, name="t1", tag="mask_s", bufs=2)
            nc.vector.tensor_tensor(t1[:], a_jb[:], u_c[:], AL.mult)
            nc.vector.tensor_scalar(t1[:], t1[:], a_i[:, sc:sc + 1], None, AL.mult)
            nc.vector.tensor_scalar(t1[:], t1[:], -1.0, 1.0, AL.mult, AL.add)
            nc.vector.tensor_tensor(t1[:], t1[:], pad_jb[:], AL.mult)
            nc.vector.tensor_scalar(t1[:], t1[:], pad_i[:, sc:sc + 1], None, AL.mult)
            # masked entries get -80: exp(-80) is a normal f32 and the masked
            # leakage (~e^-65 relative) rounds to exactly 0 in bf16 probs.
            nc.vector.tensor_scalar(
                attn_bias[:, sc, :], t1[:], 80.0, -80.0, AL.mult, AL.add
            )
        # rows with pad_i == 0 must come out as the uniform 1/S distribution
        # (reference: -1e9 bias absorbs the scores); b_fix = (1 - pad_i)/S
        b_fix = const.tile([P, SC], F32, name="b_fix")
        nc.vector.tensor_scalar(
            b_fix[:], pad_i[:], -1.0 / S, 1.0 / S, AL.mult, AL.add
        )

        # =================================================================
        # Embedding: gather + pos/type + LN  -> hT (feature-major)
        # =================================================================
        embg_b = wpool.tile([P, H], F32, name="embg_b", tag="w")
        nc.sync.dma_start(out=embg_b[:], in_=d["embg"][None, :].to_broadcast([P, H]))
        embb_b = wpool.tile([P, H], F32, name="embb_b", tag="w")
        nc.sync.dma_start(out=embb_b[:], in_=d["embb"][None, :].to_broadcast([P, H]))

        for sc in range(SC):
            idx_c = spool.tile([P, 1], I32, name="idx_c", tag="idx")
            nc.sync.dma_start(out=idx_c[:], in_=d["ids"][sc * P:(sc + 1) * P, None])
            g_c = spool.tile([P, H], F32, name="g_c", tag="tokh", bufs=2)
            nc.gpsimd.indirect_dma_start(
                out=g_c[:], out_offset=None, in_=d["wemb"][:],
                in_offset=IndirectOffsetOnAxis(ap=idx_c[:, :1], axis=0),
            )
            pt_c = spool.tile([P, H], F32, name="pt_c", tag="tokh_pt", bufs=2)
            nc.sync.dma_start(out=pt_c[:], in_=d["pt"][sc * P:(sc + 1) * P, :])
            nc.vector.tensor_tensor(g_c[:], g_c[:], pt_c[:], AL.add)

            # token-major LN over the free dim
            s1 = spool.tile([P, 1], F32, name="s1", tag="stat")
            nc.vector.reduce_sum(s1[:], g_c[:], axis=AX.X)
            mu = spool.tile([P, 1], F32, name="mu", tag="stat")
            nc.vector.tensor_scalar(mu[:], s1[:], 1.0 / H, None, AL.mult)
            sq_c = spool.tile([P, H], F32, name="sq_c", tag="tokh_sq", bufs=2)
            s2 = spool.tile([P, 1], F32, name="s2", tag="stat")
            nc.scalar.activation(sq_c[:], g_c[:], AF.Square, accum_out=s2[:])
            var = spool.tile([P, 1], F32, name="var", tag="stat")
            nc.vector.tensor_scalar(var[:], s2[:], 1.0 / H, None, AL.mult)
            mu2 = spool.tile([P, 1], F32, name="mu2", tag="stat")
            nc.vector.tensor_tensor(mu2[:], mu[:], mu[:], AL.mult)
            nc.vector.tensor_tensor(var[:], var[:], mu2[:], AL.subtract)
            sd = spool.tile([P, 1], F32, name="sd", tag="stat")
            nc.scalar.activation(sd[:], var[:], AF.Sqrt, bias=eps_col[:, :1])
            rstd = spool.tile([P, 1], F32, name="rstd", tag="stat")
            nc.vector.reciprocal(rstd[:], sd[:])

            # overwrite the (now dead) squares buffer with xhat
            xhat = sq_c
            nc.vector.tensor_scalar(
                xhat[:], g_c[:], mu[:, :1], rstd[:, :1], AL.subtract, AL.mult
            )
            nc.vector.tensor_tensor(xhat[:], xhat[:], embg_b[:], AL.mult)
            nc.vector.tensor_tensor(xhat[:], xhat[:], embb_b[:], AL.add)

            # transpose to feature-major
            for kf in range(KF):
                ps_t = psum.tile([P, 512], F32, name="ps", tag="ps")
                nc.tensor.transpose(
                    ps_t[:, :P], xhat[:, kf * P:(kf + 1) * P], ident[:]
                )
                nc.scalar.activation(
                    hT[:, kf, sc * P:(sc + 1) * P], ps_t[:, :P], AF.Identity
                )

        # =================================================================
        # Transformer layers
        # =================================================================
        for l in range(L):
            _layer(nc, c, d, l, dict(
                hT=hT, hT_bf=hT_bf, qT=qT, kT=kT, v_tok=v_tok, ctxT=ctxT,
                attn_bias=attn_bias, pad_i=pad_i, b_fix=b_fix,
                ident=ident, ident_bf=ident_bf,
                ones_col=ones_col, ones_col_bf=ones_col_bf,
                ones_row=ones_row, eps_col=eps_col,
                wpool=wpool, w2pool=w2pool, bpool=bpool, spool=spool,
                ppool=ppool, psum=psum,
            ))

        # =================================================================
        # Vocab projection: out[s, v] = h[s] @ cls_w + cls_b  (token-major)
        # =================================================================
        clsw_r = d["clsw"].rearrange("(ko p) v -> p ko v", p=P)
        out_r = d["out"].rearrange("(c p) v -> p c v", p=P)

        # final activations in bf16 for the vocab matmul
        for kf in range(KF):
            nc.vector.tensor_copy(out=hT_bf[:, kf, :], in_=hT[:, kf, :])
        for v0, vl in _nslices(V, 512):
            cw = wpool.tile([P, KF, 512], BF16, name="cw", tag="w")
            nc.sync.dma_start(out=cw[:, :, :vl], in_=clsw_r[:, :, v0:v0 + vl])
            clsb_sl = spool.tile([1, 512], F32, name="clsb_sl", tag="clsb", bufs=2)
            nc.sync.dma_start(out=clsb_sl[:, :vl], in_=d["clsb"][None, v0:v0 + vl])
            # bias broadcast tile for this slice (via SBUF: a tensor_tensor
            # may read at most one PSUM operand)
            ps_b = psum.tile([P, 512], F32, name="ps_b", tag="ps")
            nc.tensor.matmul(
                ps_b[:, :vl], lhsT=ones_row[:], rhs=clsb_sl[:, :vl],
                start=True, stop=True,
            )
            bias_bc = opool.tile([P, 512], F32, name="bias_bc", tag="bias_bc", bufs=2)
            nc.scalar.activation(bias_bc[:, :vl], ps_b[:, :vl], AF.Identity)
            for sc in range(SC):
                ps_o = psum.tile([P, 512], F32, name="ps_o", tag="ps")
                for kf in range(KF):
                    nc.tensor.matmul(
                        ps_o[:, :vl],
                        lhsT=hT_bf[:, kf, sc * P:(sc + 1) * P],
                        rhs=cw[:, kf, :vl],
                        start=(kf == 0), stop=(kf == KF - 1),
                    )
                o_sb = opool.tile([P, 512], F32, name="o_sb", tag="o")
                nc.vector.tensor_tensor(
                    o_sb[:, :vl], ps_o[:, :vl], bias_bc[:, :vl], AL.add
                )
                nc.sync.dma_start(
                    out=out_r[:, sc, v0:v0 + vl], in_=o_sb[:, :vl]
                )


def _layer(nc, c, d, l, t):
    S, H, NH, FF = c["S"], c["H"], c["NH"], c["FF"]
    DH, KF, SC, KFF, HPG = c["DH"], c["KF"], c["SC"], c["KFF"], c["HPG"]
    AL = mybir.AluOpType
    AF = mybir.ActivationFunctionType
    AX = mybir.AxisListType

    hT, qT, kT, v_tok, ctxT = (
        t["hT"], t["qT"], t["kT"], t["v_tok"], t["ctxT"]
    )
    hT_bf = t["hT_bf"]
    attn_bias, ident, ident_bf = t["attn_bias"], t["ident"], t["ident_bf"]
    pad_i, b_fix = t["pad_i"], t["b_fix"]
    ones_col, ones_row = t["ones_col"], t["ones_row"]
    ones_col_bf = t["ones_col_bf"]
    eps_col = t["eps_col"]
    wpool, w2pool, bpool = t["wpool"], t["w2pool"], t["bpool"]
    spool, ppool, psum = t["spool"], t["ppool"], t["psum"]

    # ---- helpers --------------------------------------------------------
    def wslice(w2d_ap, n0, nl, name):
        """Stream a [H, n0:n0+nl] weight slice as [128, KF, nl] (f-major)."""
        w = wpool.tile([P, KF, 512], BF16, name=name, tag="w")
        nc.sync.dma_start(
            out=w[:, :, :nl],
            in_=w2d_ap.rearrange("(ko p) n -> p ko n", p=P)[:, :, n0:n0 + nl],
        )
        return w

    def bcol(src_ap, name, scale=None):
        b = bpool.tile([P, KF], F32, name=name, tag="bcol")
        nc.sync.dma_start(out=b[:], in_=src_ap.rearrange("(ko p) -> p ko", p=P))
        if scale is not None:
            nc.vector.tensor_scalar(b[:], b[:], scale, None, AL.mult)
        return b

    with nc.named_scope("qkv"):
        # ======================================================================
        # q/k feature-major, v token-major
        # ======================================================================
        for kf in range(KF):
            nc.vector.tensor_copy(out=hT_bf[:, kf, :], in_=hT[:, kf, :])
        bq = bcol(d["ab"][l, 0], "bq", scale=1.0 / float(np.sqrt(DH)))
        for n0, nl in _nslices(H, 512):
            wq = wslice(d["aw"][l, 0], n0, nl, "wq")
            for msub in range(nl // P):
                m = n0 // P + msub
                ps = psum.tile([P, 512], F32, name="ps_qk", tag="ps")
                for kf in range(KF):
                    nc.tensor.matmul(
                        ps[:, :S],
                        lhsT=wq[:, kf, msub * P:(msub + 1) * P], rhs=hT_bf[:, kf, :],
                        start=(kf == 0), stop=(kf == KF - 1),
                    )
                nc.scalar.activation(
                    qT[:, m, :], ps[:, :S], AF.Identity,
                    bias=bq[:, m:m + 1], scale=1.0 / float(np.sqrt(DH)),
                )

        bk = bcol(d["ab"][l, 1], "bk")
        for n0, nl in _nslices(H, 512):
            wk = wslice(d["aw"][l, 1], n0, nl, "wk")
            for msub in range(nl // P):
                m = n0 // P + msub
                ps = psum.tile([P, 512], F32, name="ps_qk", tag="ps")
                for kf in range(KF):
                    nc.tensor.matmul(
                        ps[:, :S],
                        lhsT=wk[:, kf, msub * P:(msub + 1) * P], rhs=hT_bf[:, kf, :],
                        start=(kf == 0), stop=(kf == KF - 1),
                    )
                nc.scalar.activation(
                    kT[:, m, :], ps[:, :S], AF.Identity, bias=bk[:, m:m + 1]
                )

        bv_row = bpool.tile([1, H], F32, name="bv_row", tag="brow")
        nc.sync.dma_start(out=bv_row[:], in_=d["ab"][l, 2][None, :])
        for n0, nl in _nslices(H, 512):
            wv = wslice(d["aw"][l, 2], n0, nl, "wv")
            for sc in range(SC):
                ps = psum.tile([P, 512], F32, name="ps_v", tag="ps")
                for kf in range(KF):
                    nc.tensor.matmul(
                        ps[:, :nl],
                        lhsT=hT_bf[:, kf, sc * P:(sc + 1) * P],
                        rhs=wv[:, kf, :nl],
                        start=(kf == 0), stop=False,
                    )
                nc.tensor.matmul(
                    ps[:, :nl], lhsT=ones_row[:], rhs=bv_row[:, n0:n0 + nl],
                    start=False, stop=True,
                )
                nc.scalar.activation(
                    v_tok[:, sc, n0:n0 + nl], ps[:, :nl], AF.Identity
                )

    with nc.named_scope("attn"):
        # ======================================================================
        # attention per head
        # ======================================================================
        ctx_psums = {}
        for h in range(NH):
            kf_h = h // HPG
            p0 = (h % HPG) * DH
            q_h = qT[p0:p0 + DH, kf_h, :]
            k_h = kT[p0:p0 + DH, kf_h, :]

            probs = [
                ppool.tile([P, S], BF16, name=f"probs{ic}", tag="probs", bufs=8)
                for ic in range(SC)
            ]
            for ic in range(SC):
                ps_s = psum.tile([P, 512], F32, name="ps_s", tag="ps")
                nc.tensor.matmul(
                    ps_s[:, :S], lhsT=q_h[:, ic * P:(ic + 1) * P], rhs=k_h,
                    start=True, stop=False,
                )
                # mask bias folded in on the PE: psum += I.T @ bias
                nc.tensor.matmul(
                    ps_s[:, :S], lhsT=ident_bf[:], rhs=attn_bias[:, ic, :],
                    start=False, stop=True,
                )
                e_sb = spool.tile([P, S], F32, name="e_sb", tag="row_s", bufs=6)
                rowsum = spool.tile([P, 1], F32, name="rowsum", tag="stat")
                nc.scalar.activation(
                    e_sb[:], ps_s[:, :S], AF.Exp, accum_out=rowsum[:]
                )
                recip = spool.tile([P, 1], F32, name="recip", tag="stat")
                nc.vector.reciprocal(recip[:], rowsum[:])
                a_fix = spool.tile([P, 1], F32, name="a_fix", tag="stat")
                nc.vector.tensor_tensor(
                    a_fix[:], recip[:], pad_i[:, ic:ic + 1], AL.mult
                )
                nc.vector.tensor_scalar(
                    probs[ic][:], e_sb[:], a_fix[:, :1], b_fix[:, ic:ic + 1],
                    AL.mult, AL.add,
                )

            # transpose probs -> probsT (feature j on partitions)
            probsT = [
                ppool.tile([P, S], BF16, name=f"probsT{jc}", tag="probsT", bufs=8)
                for jc in range(SC)
            ]
            for jc in range(SC):
                ps_t = psum.tile([P, 512], BF16, name="ps_t", tag="ps")
                for ic in range(SC):
                    nc.tensor.transpose(
                        ps_t[:, ic * P:(ic + 1) * P],
                        probs[ic][:, jc * P:(jc + 1) * P], ident_bf[:],
                    )
                if jc % 2 == 0:
                    nc.scalar.activation(probsT[jc][:], ps_t[:, :S], AF.Identity)
                else:
                    nc.vector.tensor_copy(out=probsT[jc][:], in_=ps_t[:, :S])

            # ctx feature-major: a pair of heads shares one psum bank
            if h % HPG == 0:
                ps_c = psum.tile([P, 512], F32, name="ps_c", tag="ps")
                ctx_psums[kf_h] = ps_c
            ps_c = ctx_psums[kf_h]
            for jc in range(SC):
                nc.tensor.matmul(
                    ps_c[p0:p0 + DH, :S],
                    lhsT=v_tok[:, jc, h * DH:(h + 1) * DH],
                    rhs=probsT[jc][:],
                    start=(jc == 0), stop=(jc == SC - 1),
                    tile_position=(0, p0) if p0 else None,
                )
            if h % HPG == HPG - 1:
                nc.scalar.activation(ctxT[:, kf_h, :], ps_c[:, :S], AF.Identity)
                del ctx_psums[kf_h]

    with nc.named_scope("oproj"):
        # ======================================================================
        # attention out projection + residual + LN1
        # ======================================================================
        bo = bcol(d["ab"][l, 3], "bo")
        for n0, nl in _nslices(H, 512):
            wo = wslice(d["aw"][l, 3], n0, nl, "wo")
            for msub in range(nl // P):
                m = n0 // P + msub
                ps = psum.tile([P, 512], F32, name="ps_o", tag="ps")
                for kf in range(KF):
                    nc.tensor.matmul(
                        ps[:, :S],
                        lhsT=wo[:, kf, msub * P:(msub + 1) * P], rhs=ctxT[:, kf, :],
                        start=(kf == 0), stop=(kf == KF - 1),
                    )
                a_sb = spool.tile([P, S], F32, name="a_sb", tag="row_s", bufs=6)
                nc.scalar.activation(a_sb[:], ps[:, :S], AF.Identity, bias=bo[:, m:m + 1])
                nc.vector.tensor_tensor(hT[:, m, :], hT[:, m, :], a_sb[:], AL.add)

    _ln_feature_major(nc, c, hT, d["l1g"][l], d["l1b"][l], t, "ln1")

    with nc.named_scope("ffn"):
        # ======================================================================
        # FFN (blocked over FF so u never lives whole)
        # ======================================================================
        for kf in range(KF):
            nc.vector.tensor_copy(out=hT_bf[:, kf, :], in_=hT[:, kf, :])
        b1 = bpool.tile([P, KFF], F32, name="b1", tag="b1col")
        nc.sync.dma_start(out=b1[:], in_=d["b1"][l].rearrange("(ko p) -> p ko", p=P))
        b2 = bcol(d["b2"][l], "b2")
        w1_r = d["w1"][l].rearrange("(ko p) n -> p ko n", p=P)
        w2_r = d["w2"][l].rearrange("(ko p) n -> p ko n", p=P)
        ps_d = [
            psum.tile([P, 512], F32, name=f"ps_d{m}", tag="ps") for m in range(KF)
        ]
        for b0, bl in _nslices(FF, 512):
            w1s = wpool.tile([P, KF, 512], BF16, name="w1s", tag="w")
            nc.sync.dma_start(out=w1s[:, :, :bl], in_=w1_r[:, :, b0:b0 + bl])
            ublk = spool.tile([P, 4, S], BF16, name="ublk", tag="ublk", bufs=2)
            for j in range(bl // P):
                kff = b0 // P + j
                ps_u = psum.tile([P, 512], F32, name="ps_u", tag="ps")
                for kf in range(KF):
                    nc.tensor.matmul(
                        ps_u[:, :S],
                        lhsT=w1s[:, kf, j * P:(j + 1) * P], rhs=hT_bf[:, kf, :],
                        start=(kf == 0), stop=(kf == KF - 1),
                    )
                nc.scalar.activation(
                    ublk[:, j, :], ps_u[:, :S], AF.Gelu, bias=b1[:, kff:kff + 1]
                )
            for j in range(bl // P):
                kff = b0 // P + j
                w2c = w2pool.tile([P, H], BF16, name="w2c", tag="w2")
                nc.sync.dma_start(out=w2c[:], in_=w2_r[:, kff, :])
                for m in range(KF):
                    nc.tensor.matmul(
                        ps_d[m][:, :S],
                        lhsT=w2c[:, m * P:(m + 1) * P], rhs=ublk[:, j, :],
                        start=(kff == 0), stop=(kff == KFF - 1),
                    )
        for m in range(KF):
            d_sb = spool.tile([P, S], F32, name="d_sb", tag="row_s", bufs=6)
            nc.scalar.activation(d_sb[:], ps_d[m][:, :S], AF.Identity, bias=b2[:, m:m + 1])
            nc.vector.tensor_tensor(hT[:, m, :], hT[:, m, :], d_sb[:], AL.add)

    _ln_feature_major(nc, c, hT, d["l2g"][l], d["l2b"][l], t, "ln2")


def _ln_feature_major(nc, c, hT, g_dram, b_dram, t, name):
    """LayerNorm over the feature (partition) dim of feature-major hT, in place."""
    import contextlib
    _sc = contextlib.ExitStack()
    _sc.enter_context(nc.named_scope("ln"))
    S, H, KF = c["S"], c["H"], c["KF"]
    AL = mybir.AluOpType
    AF = mybir.ActivationFunctionType
    ones_col, ones_row = t["ones_col"], t["ones_row"]
    eps_col = t["eps_col"]
    spool, bpool, psum = t["spool"], t["bpool"], t["psum"]

    hT_bf = t["hT_bf"]
    ones_col_bf = t["ones_col_bf"]
    g_sb = bpool.tile([P, KF], mybir.dt.float32, name=f"{name}_g", tag="bcol")
    nc.sync.dma_start(out=g_sb[:], in_=g_dram.rearrange("(ko p) -> p ko", p=P))
    b_sb = bpool.tile([P, KF], mybir.dt.float32, name=f"{name}_b", tag="bcol")
    nc.sync.dma_start(out=b_sb[:], in_=b_dram.rearrange("(ko p) -> p ko", p=P))

    # stats from the bf16 copy: S1 = sum_f h, S2 = sum_f h^2. The per-element
    # bf16 rounding noise averages out over H; mean/var error is O(1e-4).
    for kf in range(KF):
        nc.vector.tensor_copy(out=hT_bf[:, kf, :], in_=hT[:, kf, :])
    ps_s1 = psum.tile([P, 512], F32, name=f"{name}_s1", tag="ps")
    ps_s2 = psum.tile([P, 512], F32, name=f"{name}_s2", tag="ps")
    for kf in range(KF):
        nc.tensor.matmul(
            ps_s1[:1, :S], lhsT=ones_col_bf[:, :1], rhs=hT_bf[:, kf, :],
            start=(kf == 0), stop=(kf == KF - 1),
        )
        sq = spool.tile([P, S], BF16, name=f"{name}_sq", tag="row_sbf")
        nc.scalar.activation(sq[:], hT_bf[:, kf, :], AF.Square)
        nc.tensor.matmul(
            ps_s2[:1, :S], lhsT=ones_col_bf[:, :1], rhs=sq[:],
            start=(kf == 0), stop=(kf == KF - 1),
        )

    mu = spool.tile([1, S], F32, name=f"{name}_mu", tag="lrow", bufs=4)
    nc.vector.tensor_scalar(mu[:], ps_s1[:1, :S], 1.0 / H, None, AL.mult)
    e2 = spool.tile([1, S], F32, name=f"{name}_e2", tag="lrow", bufs=4)
    nc.vector.tensor_scalar(e2[:], ps_s2[:1, :S], 1.0 / H, None, AL.mult)
    var = spool.tile([1, S], F32, name=f"{name}_var", tag="lrow", bufs=4)
    nc.vector.tensor_tensor(var[:], mu[:], mu[:], AL.mult)
    nc.vector.tensor_tensor(var[:], e2[:], var[:], AL.subtract)
    sd = spool.tile([1, S], F32, name=f"{name}_sd", tag="lrow", bufs=4)
    nc.scalar.activation(sd[:], var[:], AF.Sqrt, bias=eps_col[:1, :1])
    rstd = spool.tile([1, S], F32, name=f"{name}_rstd", tag="lrow", bufs=4)
    nc.vector.reciprocal(rstd[:], sd[:])
    # negated mu*rstd so the LN apply can use commutative ops with the
    # PSUM operand in slot 0 (a tensor_tensor may only read PSUM via in0)
    mrs = spool.tile([1, S], F32, name=f"{name}_mrs", tag="lrow", bufs=4)
    nc.vector.tensor_tensor(mrs[:], mu[:], rstd[:], AL.mult)
    nc.vector.tensor_scalar(mrs[:], mrs[:], -1.0, None, AL.mult)

    # broadcast rstd / mu*rstd across partitions (full fp32 matmuls)
    ps_r = psum.tile([P, 512], F32, name=f"{name}_br", tag="ps")
    ps_m = psum.tile([P, 512], F32, name=f"{name}_bm", tag="ps")
    for s0, sl in _nslices(S):
        nc.tensor.matmul(ps_r[:, s0:s0 + sl],
                         lhsT=ones_row[:], rhs=rstd[:, s0:s0 + sl],
                         start=(s0 == 0), stop=(s0 + sl >= S))
        nc.tensor.matmul(ps_m[:, s0:s0 + sl],
                         lhsT=ones_row[:], rhs=mrs[:, s0:s0 + sl],
                         start=(s0 == 0), stop=(s0 + sl >= S))

    for kf in range(KF):
        tt = spool.tile([P, S], F32, name=f"{name}_t", tag="row_s", bufs=6)
        nc.vector.tensor_tensor(tt[:], ps_r[:, :S], hT[:, kf, :], AL.mult)
        nc.vector.tensor_tensor(tt[:], ps_m[:, :S], tt[:], AL.add)
        nc.vector.tensor_scalar(
            hT[:, kf, :], tt[:], g_sb[:, kf:kf + 1], b_sb[:, kf:kf + 1],
            AL.mult, AL.add,
        )
    _sc.close()


# =========================================================================
# Host entry point
# =========================================================================

_NC_CACHE = {}


def _get_nc():
    key = "full"
    if key not in _NC_CACHE:
        _NC_CACHE[key] = build_nc(FULL_CFG)
    return _NC_CACHE[key]


def _prep_in_maps(inputs):
    import ml_dtypes

    cfg = FULL_CFG
    B = N_CORES
    ids = np.asarray(inputs["input_ids"], dtype=np.int32)
    assert ids.shape == (B, cfg["S"])

    pos_type = (
        np.asarray(inputs["pos_emb"], np.float32)
        + np.asarray(inputs["type_emb"], np.float32)[0][None, :]
    )
    bf = lambda k: np.ascontiguousarray(
        np.asarray(inputs[k], np.float32).astype(ml_dtypes.bfloat16)
    )

    shared = {
        "word_emb": np.ascontiguousarray(inputs["word_emb"], np.float32),
        "pos_type_emb": np.ascontiguousarray(pos_type, np.float32),
        "emb_ln_g": np.ascontiguousarray(inputs["emb_ln_g"], np.float32),
        "emb_ln_b": np.ascontiguousarray(inputs["emb_ln_b"], np.float32),
        "attn_w": bf("attn_w"),
        "attn_b": np.ascontiguousarray(inputs["attn_b"], np.float32),
        "ln1_g": np.ascontiguousarray(inputs["ln1_g"], np.float32),
        "ln1_b": np.ascontiguousarray(inputs["ln1_b"], np.float32),
        "ffn_w1": bf("ffn_w1"),
        "ffn_b1": np.ascontiguousarray(inputs["ffn_b1"], np.float32),
        "ffn_w2": bf("ffn_w2"),
        "ffn_b2": np.ascontiguousarray(inputs["ffn_b2"], np.float32),
        "ln2_g": np.ascontiguousarray(inputs["ln2_g"], np.float32),
        "ln2_b": np.ascontiguousarray(inputs["ln2_b"], np.float32),
        "cls_w": bf("cls_w"),
        "cls_b": np.ascontiguousarray(inputs["cls_b"], np.float32),
    }
    in_maps = [
        {"input_ids": np.ascontiguousarray(ids[i]), **shared} for i in range(B)
    ]
    return in_maps


def _run(inputs, trace=False, **kw):
    from concourse.bass_utils import run_bass_kernel_spmd

    in_maps = _prep_in_maps(inputs)
    nc = _get_nc()
    res = run_bass_kernel_spmd(nc, in_maps, list(range(N_CORES)), trace=trace, **kw)
    out = np.stack(
        [res.results[i]["out"] for i in range(N_CORES)], axis=0
    ).astype(np.float32)
    return out, res


def kernel(**inputs):
    out, _ = _run(inputs, trace=False)
    return out


def run_traced(**inputs):
    return _run(inputs, trace=True)



# revision 27
# speedup vs baseline: 2.9681x; 2.9681x over previous
"""Trainium2 Bass kernel for a 12-layer BERT generator model.

Model (see problem reference):
  B=8, S=512, H=768, L=12, NH=12 (DH=64), FF=3072, V=21128
  - embedding gather + pos/type embeddings + LN
  - L x { QA-causal masked multi-head attention + LN, exact-GELU FFN + LN }
  - vocab projection [S, V]

Sharding: data-parallel, one sample per NeuronCore (B == n_cores == 8).

Design notes (v2, "transposed attention"):
  - activations FEATURE-major in SBUF: hT[p, kf, s] = h[s, kf*128+p].
  - attention scores are computed TRANSPOSED (keys on partitions, queries on
    the free dim): scoresT[j, i] = sum_d k[j,d] q[i,d].  Softmax reduction is
    then a PARTITION reduction done off the critical PE path: exp on ACT,
    per-key-chunk sums added on the Pool engine, partition_all_reduce on
    Pool, reciprocal on DVE, and the context is normalized per head with one
    DVE multiply.  This removes all PE prob-transposes of the baseline.
  - padded queries: reference gives them exactly uniform attention.  We zero
    the q columns of padded tokens and give those columns a uniform -80 bias,
    so exp() is constant along keys and normalization yields exactly 1/S.
  - fp32 matmuls avoided everywhere (4x PE cost): broadcast matmuls use
    float32r (fp22) bitcasts; all GEMMs are bf16.
  - LayerNorm: stats (S1=sum h, S2=sum h^2 over features) accumulate on the
    PE interleaved with the producing projection; rstd = exp(-0.5*ln(var))
    so ACT stays within the exp/ln table family (no sqrt table loads).
  - weights are host-repacked so every weight DMA is per-partition
    contiguous (6-9KB runs).
"""

import sys

sys.path.insert(0, "/opt/trn_rl_repo")

import numpy as np

import concourse.bass as bass
import concourse.mybir as mybir
import concourse.tile as tile
from concourse import bacc
from concourse.bass import IndirectOffsetOnAxis
from concourse.bass_isa import ReduceOp
from concourse.masks import make_identity

P = 128
PAD_ID, SEP_ID = 0, 102
EPS = 1e-12

F32 = mybir.dt.float32
F32R = mybir.dt.float32r
BF16 = mybir.dt.bfloat16
F16 = mybir.dt.float16
I32 = mybir.dt.int32

FULL_CFG = dict(S=512, H=768, NH=12, L=12, FF=3072, V=21128)
N_CORES = 8
V_PAD = 21504  # 42 * 512


def _r(ap):
    """Reinterpret an fp32 AP as float32r for fast (1 cyc/row) matmul."""
    return ap.bitcast(F32R)


def build_nc(cfg, debug=False):
    S, H, NH, L, FF, V = (
        cfg["S"], cfg["H"], cfg["NH"], cfg["L"], cfg["FF"], cfg["V"],
    )
    DH = 64
    assert H % P == 0 and S % P == 0 and FF % P == 0
    assert H // NH == DH
    KF = H // P          # feature subtiles (6)
    SC = S // P          # token chunks (4)
    KFF = FF // P        # ffn subtiles (24)
    NVB = V_PAD // 512   # vocab 512-blocks (42)

    nc = bacc.Bacc("TRN2", target_bir_lowering=False, debug=debug)

    # ---- I/O ------------------------------------------------------------
    ids_d = nc.declare_dram_parameter("input_ids", [S], I32, False)
    wemb_d = nc.declare_dram_parameter("word_emb", [V, H], F32, False)
    pt_d = nc.declare_dram_parameter("pos_type_emb", [S, H], F32, False)

    # repacked: aw_x[l, i, p, ko, n] = attn_w[l, i, ko*128+p, n]
    aw_d = nc.declare_dram_parameter("attn_wx", [L, 4, P, KF, H], BF16, False)
    ab_d = nc.declare_dram_parameter("attn_b", [L, 4, H], F32, False)
    l1g_d = nc.declare_dram_parameter("ln1_g", [L, H], F32, False)
    gprev_d = nc.declare_dram_parameter("gprev", [L, H], F32, False)
    # w1x[l, b, p, ko, j] = ffn_w1[l, ko*128+p, b*512+j]
    w1_d = nc.declare_dram_parameter("ffn_w1x", [L, FF // 512, P, KF, 512], BF16, False)
    b1_d = nc.declare_dram_parameter("ffn_b1", [L, FF], F32, False)
    # w2x[l, m, p, ko, j] = ffn_w2[l, ko*128+p, m*128+j]
    w2_d = nc.declare_dram_parameter("ffn_w2x", [L, KF, P, KFF, P], BF16, False)
    b2_d = nc.declare_dram_parameter("ffn_b2", [L, H], F32, False)

    # clsx[vb, p, ko, n] = cls_w_padded[ko*128+p, vb*512+n]
    clsw_d = nc.declare_dram_parameter("cls_wx", [NVB, P, KF, 512], BF16, False)
    clsb_d = nc.declare_dram_parameter("cls_b", [V_PAD], BF16, False)
    out_d = nc.declare_dram_parameter("out", [S, V], F32, True)

    with tile.TileContext(nc) as tc:
        _build_body(
            nc, tc,
            dict(S=S, H=H, NH=NH, L=L, FF=FF, V=V, DH=DH, KF=KF, SC=SC,
                 KFF=KFF, NVB=NVB),
            dict(ids=ids_d, wemb=wemb_d, pt=pt_d,
                 aw=aw_d, ab=ab_d, l1g=l1g_d, gprev=gprev_d, w1=w1_d, b1=b1_d,
                 w2=w2_d, b2=b2_d, clsw=clsw_d,
                 clsb=clsb_d, out=out_d),
        )
    nc.compile()
    return nc


def _build_body(nc, tc, c, d):
    S, H, NH, L, FF, V = c["S"], c["H"], c["NH"], c["L"], c["FF"], c["V"]
    DH, KF, SC, KFF, NVB = c["DH"], c["KF"], c["SC"], c["KFF"], c["NVB"]
    AL = mybir.AluOpType
    AF = mybir.ActivationFunctionType
    AX = mybir.AxisListType

    import contextlib

    ctx = contextlib.ExitStack()
    with ctx:
        const = ctx.enter_context(tc.tile_pool(name="const", bufs=1))
        persist = ctx.enter_context(tc.tile_pool(name="persist", bufs=1))
        wpool = ctx.enter_context(tc.tile_pool(name="wpool", bufs=1))
        bpool = ctx.enter_context(tc.tile_pool(name="bpool", bufs=2))
        spool = ctx.enter_context(tc.tile_pool(name="spool", bufs=3))
        psum = ctx.enter_context(tc.tile_pool(name="psum", bufs=1, space="PSUM"))

        # ---- constants --------------------------------------------------
        ident_bf = const.tile([P, P], BF16, name="ident_bf")
        make_identity(nc, ident_bf)
        ones_col_bf = const.tile([P, 1], BF16, name="ones_col_bf")
        nc.gpsimd.memset(ones_col_bf[:], 1.0)
        ones_row = const.tile([1, P], BF16, name="ones_row")  # K=1 bcast lhsT
        nc.gpsimd.memset(ones_row[:], 1.0)
        ones_row_f = const.tile([1, P], F32, name="ones_row_f")
        nc.gpsimd.memset(ones_row_f[:], 1.0)
        eps_col = const.tile([P, 1], F32, name="eps_col")
        nc.gpsimd.memset(eps_col[:], EPS)


        # ---- persistent activations ------------------------------------
        hT = persist.tile([P, KF, S], F32, name="hT")
        hT_bf = persist.tile([P, KF, S], BF16, name="hT_bf")
        qT = persist.tile([P, KF, S], BF16, name="qT")
        kT = persist.tile([P, KF, S], BF16, name="kT")
        v_tok = persist.tile([P, SC, H], BF16, name="v_tok")
        ctxT = persist.tile([P, KF, S], BF16, name="ctxT")
        ublk = persist.tile([P, KFF, S], BF16, name="ublk")
        attn_biasT = persist.tile([P, SC, S], BF16, name="attn_biasT")
        pad_q_bf = persist.tile([P, S], BF16, name="pad_q_bf")

        # =================================================================
        # Mask: transposed additive attention bias from input_ids
        #   attn_biasT[j - partition, jc, i - free] = 0 if mask[i, j] else -80
        # =================================================================
        ids_row_i = spool.tile([1, S], I32, name="ids_row_i")
        nc.sync.dma_start(out=ids_row_i[:], in_=d["ids"][None, :])
        ids_row = const.tile([1, S], F32, name="ids_row")
        nc.vector.tensor_copy(out=ids_row[:], in_=ids_row_i[:])

        ids_p_i = spool.tile([P, SC], I32, name="ids_p_i")
        nc.sync.dma_start(
            out=ids_p_i[:], in_=d["ids"].rearrange("(c p) -> p c", p=P)
        )
        ids_p = const.tile([P, SC], F32, name="ids_p")
        nc.vector.tensor_copy(out=ids_p[:], in_=ids_p_i[:])

        iota_j_i = spool.tile([1, S], I32, name="iota_j_i")
        nc.gpsimd.iota(iota_j_i[:], pattern=[[1, S]], base=0, channel_multiplier=0)
        iota_j = const.tile([1, S], F32, name="iota_j")
        nc.vector.tensor_copy(out=iota_j[:], in_=iota_j_i[:])

        iota_p_i = spool.tile([P, SC], I32, name="iota_p_i")
        nc.gpsimd.iota(iota_p_i[:], pattern=[[P, SC]], base=0, channel_multiplier=1)
        iota_p = const.tile([P, SC], F32, name="iota_p")
        nc.vector.tensor_copy(out=iota_p[:], in_=iota_p_i[:])

        pad_f = const.tile([1, S], F32, name="pad_f")      # pad over free dim
        nc.vector.tensor_scalar(pad_f[:], ids_row[:], float(PAD_ID), None, AL.not_equal)
        pad_p = const.tile([P, SC], F32, name="pad_p")     # pad over partitions
        nc.vector.tensor_scalar(pad_p[:], ids_p[:], float(PAD_ID), None, AL.not_equal)

        # first-SEP position -> qlen = pos + 1
        sep = spool.tile([1, S], F32, name="sep", tag="lrow", bufs=3)
        nc.vector.tensor_scalar(sep[:], ids_row[:], float(SEP_ID), None, AL.is_equal)
        tsel = spool.tile([1, S], F32, name="tsel", tag="lrow", bufs=3)
        nc.vector.tensor_scalar(tsel[:], iota_j[:], float(S), None, AL.subtract)
        nc.vector.tensor_tensor(tsel[:], tsel[:], sep[:], AL.mult)
        nc.vector.tensor_scalar(tsel[:], tsel[:], float(S), None, AL.add)
        qlen = const.tile([1, 1], F32, name="qlen")
        nc.vector.tensor_reduce(qlen[:], tsel[:], axis=AX.X, op=AL.min)
        nc.vector.tensor_scalar(qlen[:], qlen[:], 1.0, None, AL.add)

        # broadcast qlen to all partitions (K=1 matmul; bf16 exact <= 256)
        qlen_bf = const.tile([1, 1], BF16, name="qlen_bf")
        nc.vector.tensor_copy(out=qlen_bf[:], in_=qlen[:])
        ps_q = psum.tile([P, 512], F32, name="ps_q", tag="pp", bufs=4)
        nc.tensor.matmul(ps_q[:, :1], lhsT=ones_row[:], rhs=qlen_bf[:],
                         start=True, stop=True)
        qlen_b = const.tile([P, 1], F32, name="qlen_b")
        nc.vector.tensor_copy(out=qlen_b[:], in_=ps_q[:, :1])

        a_p = const.tile([P, SC], F32, name="a_p")   # key token in answer
        nc.vector.tensor_scalar(a_p[:], iota_p[:], qlen_b[:, :1], None, AL.is_ge)
        a_f = spool.tile([1, S], F32, name="a_f", tag="lrow", bufs=3)
        nc.vector.tensor_scalar(a_f[:], iota_j[:], qlen[:, :1], None, AL.is_ge)

        # broadcast a_f and pad_f across partitions (0/1 exact in bf16)
        a_f_bf = spool.tile([1, S], BF16, name="a_f_bf", tag="lrow_bf", bufs=2)
        nc.vector.tensor_copy(out=a_f_bf[:], in_=a_f[:])
        pad_f_bf = spool.tile([1, S], BF16, name="pad_f_bf", tag="lrow_bf", bufs=2)
        nc.vector.tensor_copy(out=pad_f_bf[:], in_=pad_f[:])
        ps_a = psum.tile([P, 512], F32, name="ps_a", tag="pp", bufs=4)
        a_fb = const.tile([P, S], F32, name="a_fb")
        nc.tensor.matmul(ps_a[:, :S], lhsT=ones_row[:], rhs=a_f_bf[:],
                         start=True, stop=True)
        nc.vector.tensor_copy(out=a_fb[:], in_=ps_a[:, :S])
        ps_p = psum.tile([P, 512], F32, name="ps_p", tag="pp", bufs=4)
        pad_fb = const.tile([P, S], F32, name="pad_fb")
        nc.tensor.matmul(ps_p[:, :S], lhsT=ones_row[:], rhs=pad_f_bf[:],
                         start=True, stop=True)
        nc.vector.tensor_copy(out=pad_fb[:], in_=ps_p[:, :S])
        nc.vector.tensor_copy(out=pad_q_bf[:], in_=pad_fb[:])

        for jc in range(SC):
            # ok[p, i] = 1.0 if i >= jc*128 + p  (query at-or-after key j)
            ok = spool.tile([P, S], F32, name="ok", tag="row_s", bufs=3)
            nc.gpsimd.memset(ok[:], 1.0)
            nc.gpsimd.affine_select(
                out=ok[:], in_=ok[:], compare_op=AL.is_gt, fill=0.0,
                base=-(jc * P) + 1, channel_multiplier=-1, pattern=[[1, S]],
            )
            # t1 = -viol = -a_q*(1 - ok);  mask = pad_q*pad_k*(1 + t1*a_k)
            t1 = spool.tile([P, S], F32, name="t1", tag="row_s", bufs=3)
            nc.vector.tensor_tensor(t1[:], a_fb[:], ok[:], AL.mult)
            nc.vector.tensor_tensor(t1[:], t1[:], a_fb[:], AL.subtract)
            nc.vector.tensor_scalar(t1[:], t1[:], a_p[:, jc:jc + 1], 1.0, AL.mult, AL.add)
            nc.vector.tensor_tensor(t1[:], t1[:], pad_fb[:], AL.mult)
            nc.vector.tensor_scalar(t1[:], t1[:], pad_p[:, jc:jc + 1], None, AL.mult)
            # masked entries -80: exp(-80) is normal in fp32/bf16 and the
            # leakage vanishes after normalization.
            nc.vector.tensor_scalar(
                attn_biasT[:, jc, :], t1[:], 80.0, -80.0, AL.mult, AL.add
            )

        # =================================================================
        # Embedding: gather + pos/type + LN (token-major) -> hT, hT_bf
        # =================================================================
        for sc in range(SC):
            idx_c = spool.tile([P, 1], I32, name="idx_c", tag="idx")
            nc.sync.dma_start(out=idx_c[:], in_=d["ids"][sc * P:(sc + 1) * P, None])
            g_c = spool.tile([P, H], F32, name="g_c", tag="tokh", bufs=2)
            nc.gpsimd.indirect_dma_start(
                out=g_c[:], out_offset=None, in_=d["wemb"][:],
                in_offset=IndirectOffsetOnAxis(ap=idx_c[:, :1], axis=0),
            )
            pt_c = spool.tile([P, H], F32, name="pt_c", tag="tokh_pt", bufs=2)
            nc.sync.dma_start(out=pt_c[:], in_=d["pt"][sc * P:(sc + 1) * P, :])
            nc.vector.tensor_tensor(g_c[:], g_c[:], pt_c[:], AL.add)

            # token-major LN over the free dim
            s1 = spool.tile([P, 1], F32, name="s1", tag="stat")
            nc.vector.reduce_sum(s1[:], g_c[:], axis=AX.X)
            mu = spool.tile([P, 1], F32, name="mu", tag="stat")
            nc.vector.tensor_scalar(mu[:], s1[:], 1.0 / H, None, AL.mult)
            sq_c = spool.tile([P, H], F32, name="sq_c", tag="tokh_pt", bufs=2)
            s2 = spool.tile([P, 1], F32, name="s2", tag="stat")
            nc.scalar.activation(sq_c[:], g_c[:], AF.Square, accum_out=s2[:])
            var = spool.tile([P, 1], F32, name="var", tag="stat")
            nc.vector.tensor_scalar(var[:], s2[:], 1.0 / H, None, AL.mult)
            mu2 = spool.tile([P, 1], F32, name="mu2", tag="stat")
            nc.vector.tensor_tensor(mu2[:], mu[:], mu[:], AL.mult)
            nc.vector.tensor_tensor(var[:], var[:], mu2[:], AL.subtract)
            sdv = spool.tile([P, 1], F32, name="sdv", tag="stat")
            nc.scalar.activation(sdv[:], var[:], AF.Sqrt, bias=eps_col[:, :1])
            rstd = spool.tile([P, 1], F32, name="rstd", tag="stat")
            nc.vector.reciprocal(rstd[:], sdv[:])

            xhat = spool.tile([P, H], BF16, name="xhat", tag="tokh_x", bufs=2)
            nc.vector.tensor_scalar(
                xhat[:], g_c[:], mu[:, :1], rstd[:, :1], AL.subtract, AL.mult
            )

            # transpose to feature-major (bf16: 1 cyc/row on the PE)
            for kf in range(KF):
                ps_t = psum.tile([P, P], BF16, name="ps_t", tag="pp", bufs=4)
                nc.tensor.transpose(
                    ps_t[:, :P], xhat[:, kf * P:(kf + 1) * P], ident_bf[:]
                )
                nc.scalar.activation(
                    hT[:, kf, sc * P:(sc + 1) * P], ps_t[:, :P], AF.Identity
                )

        for kf in range(KF):
            nc.vector.tensor_copy(out=hT_bf[:, kf, :], in_=hT[:, kf, :])

        # =================================================================
        # Transformer layers
        # =================================================================
        t = dict(
            hT=hT, hT_bf=hT_bf, qT=qT, kT=kT, v_tok=v_tok, ctxT=ctxT,
            ublk=ublk, attn_biasT=attn_biasT, pad_q_bf=pad_q_bf,
            ident_bf=ident_bf, ones_col_bf=ones_col_bf, ones_row=ones_row,
            ones_row_f=ones_row_f, eps_col=eps_col,
            wpool=wpool, bpool=bpool, spool=spool, psum=psum,
        )
        for l in range(L):
            _layer(nc, c, d, l, t)

        # =================================================================
        # Vocab projection, hT-stationary blocks of 3 v-slices
        # =================================================================
        out_r = d["out"].rearrange("(c p) v -> p c v", p=P)
        NB = NVB // 3  # 14 blocks of 3
        for vb in range(NB):
            cws = []
            biases = []
            for tt in range(3):
                vi = vb * 3 + tt
                cw = wpool.tile([P, KF, 512], BF16, name="cw", tag="w", bufs=3)
                nc.sync.dma_start(out=cw[:], in_=d["clsw"][vi])
                cws.append(cw)
                clsb_sl = spool.tile([1, 512], BF16, name="clsb_sl", tag="lrow_bf", bufs=2)
                nc.sync.dma_start(out=clsb_sl[:], in_=d["clsb"][None, vi * 512:(vi + 1) * 512])
                ps_b = psum.tile([P, 512], F32, name="ps_b", tag="pst", bufs=2)
                nc.tensor.matmul(
                    ps_b[:, :512], lhsT=ones_row[:], rhs=clsb_sl[:],
                    start=True, stop=True,
                )
                bias_bc = spool.tile([P, 512], F32, name="bias_bc", tag="row_s", bufs=3)
                nc.scalar.activation(bias_bc[:], ps_b[:, :512], AF.Identity)
                biases.append(bias_bc)
            for sc in range(SC):
                ps_os = [
                    psum.tile([P, 512], F32, name=f"ps_o{tt}", tag="pp", bufs=4)
                    for tt in range(3)
                ]
                for kf in range(KF):
                    for tt in range(3):
                        nc.tensor.matmul(
                            ps_os[tt][:, :512],
                            lhsT=hT_bf[:, kf, sc * P:(sc + 1) * P],
                            rhs=cws[tt][:, kf, :],
                            start=(kf == 0), stop=(kf == KF - 1),
                        )
                for tt in range(3):
                    vi = vb * 3 + tt
                    v0 = vi * 512
                    vl = min(512, V - v0)
                    if vl <= 0:
                        continue
                    o_sb = spool.tile([P, 512], F32, name="o_sb", tag="o", bufs=2)
                    nc.vector.tensor_tensor(
                        o_sb[:, :512], ps_os[tt][:, :512], biases[tt][:], AL.add
                    )
                    nc.sync.dma_start(
                        out=out_r[:, sc, v0:v0 + vl], in_=o_sb[:, :vl]
                    )


def _layer(nc, c, d, l, t):
    S, H, NH, FF = c["S"], c["H"], c["NH"], c["FF"]
    DH, KF, SC, KFF = c["DH"], c["KF"], c["SC"], c["KFF"]
    AL = mybir.AluOpType
    AF = mybir.ActivationFunctionType

    hT, hT_bf, qT, kT, v_tok, ctxT = (
        t["hT"], t["hT_bf"], t["qT"], t["kT"], t["v_tok"], t["ctxT"]
    )
    ublk = t["ublk"]
    attn_biasT, pad_q_bf = t["attn_biasT"], t["pad_q_bf"]
    ident_bf, ones_col_bf, ones_row = t["ident_bf"], t["ones_col_bf"], t["ones_row"]

    wpool, bpool, spool, psum = t["wpool"], t["bpool"], t["spool"], t["psum"]

    def bcol(src_ap, name, scale=None):
        b = bpool.tile([P, KF], F32, name=name, tag="bcol", bufs=4)
        nc.sync.dma_start(out=b[:], in_=src_ap.rearrange("(ko p) -> p ko", p=P))
        if scale is not None:
            nc.vector.tensor_scalar(b[:], b[:], scale, None, AL.mult)
        return b

    with nc.named_scope("qkv"):
        # ---- q/k feature-major ------------------------------------------
        wq = wpool.tile([P, KF, H], BF16, name="wq", tag="w", bufs=3)
        nc.sync.dma_start(out=wq[:], in_=d["aw"][l, 0])
        bq = bcol(d["ab"][l, 0], "bq", scale=1.0 / float(np.sqrt(DH)))
        for half in range(2):
            pss = [
                psum.tile([P, 512], F32, name=f"ps_q{mm}", tag="pp", bufs=4)
                for mm in range(3)
            ]
            for kf in range(KF):
                for mm in range(3):
                    m = half * 3 + mm
                    nc.tensor.matmul(
                        pss[mm][:, :S],
                        lhsT=wq[:, kf, m * P:(m + 1) * P], rhs=hT_bf[:, kf, :],
                        start=(kf == 0), stop=(kf == KF - 1),
                    )
            for mm in range(3):
                m = half * 3 + mm
                nc.scalar.activation(
                    qT[:, m, :], pss[mm][:, :S], AF.Identity,
                    bias=bq[:, m:m + 1], scale=1.0 / float(np.sqrt(DH)),
                )
                # zero q columns of padded tokens -> uniform attention there
                nc.vector.tensor_tensor(qT[:, m, :], qT[:, m, :], pad_q_bf[:], AL.mult)

        wk = wpool.tile([P, KF, H], BF16, name="wk", tag="w", bufs=3)
        nc.sync.dma_start(out=wk[:], in_=d["aw"][l, 1])
        bk = bcol(d["ab"][l, 1], "bk")
        for m in range(KF):
            ps = psum.tile([P, 512], F32, name="ps_qk", tag="pp", bufs=4)
            for kf in range(KF):
                nc.tensor.matmul(
                    ps[:, :S],
                    lhsT=wk[:, kf, m * P:(m + 1) * P], rhs=hT_bf[:, kf, :],
                    start=(kf == 0), stop=(kf == KF - 1),
                )
            nc.scalar.activation(
                kT[:, m, :], ps[:, :S], AF.Identity, bias=bk[:, m:m + 1]
            )

        # ---- v token-major ----------------------------------------------
        wv = wpool.tile([P, KF, H], BF16, name="wv", tag="w", bufs=3)
        nc.sync.dma_start(out=wv[:], in_=d["aw"][l, 2])
        bv_row = bpool.tile([1, H], F32, name="bv_row", tag="brow", bufs=2)
        nc.sync.dma_start(out=bv_row[:], in_=d["ab"][l, 2][None, :])
        bv_bf = bpool.tile([1, H], BF16, name="bv_bf", tag="brow_bf", bufs=2)
        nc.vector.tensor_copy(out=bv_bf[:], in_=bv_row[:])
        for n0 in (0, 512):
            nl = min(512, H - n0)
            for sc in range(SC):
                ps = psum.tile([P, 512], F32, name="ps_v", tag="pp", bufs=4)
                nc.tensor.matmul(
                    ps[:, :nl], lhsT=ones_row[:], rhs=bv_bf[:, n0:n0 + nl],
                    start=True, stop=False,
                )
                for kf in range(KF):
                    nc.tensor.matmul(
                        ps[:, :nl],
                        lhsT=hT_bf[:, kf, sc * P:(sc + 1) * P],
                        rhs=wv[:, kf, n0:n0 + nl],
                        start=False, stop=(kf == KF - 1),
                    )
                nc.scalar.activation(
                    v_tok[:, sc, n0:n0 + nl], ps[:, :nl], AF.Identity
                )

    with nc.named_scope("attn"):
        # per head: scoresT -> +biasT -> exp -> (Pool) colsum -> recip
        #           ctx (paired psum) -> normalize
        for g in range(NH // 2):
            ps_c = psum.tile([P, 512], F32, name="ps_c", tag="pc", bufs=2)
            for hh in range(2):
                h = 2 * g + hh
                p0 = hh * DH
                expT = spool.tile([P, SC, S], BF16, name="expT", tag="expT", bufs=2)
                for jc in range(SC):
                    ps_s = psum.tile([P, 512], F32, name="ps_s", tag="pp", bufs=4)
                    nc.tensor.matmul(
                        ps_s[:, :S],
                        lhsT=kT[p0:p0 + DH, g, jc * P:(jc + 1) * P],
                        rhs=qT[p0:p0 + DH, g, :],
                        start=True, stop=False,
                    )
                    nc.tensor.matmul(
                        ps_s[:, :S], lhsT=ident_bf[:], rhs=attn_biasT[:, jc, :],
                        start=False, stop=True,
                    )
                    nc.scalar.activation(expT[:, jc, :], ps_s[:, :S], AF.Exp)
                # denominator: sum over keys = partitions x 4 chunks (Pool)
                es1 = spool.tile([P, S], BF16, name="es1", tag="es", bufs=3)
                nc.vector.tensor_tensor(es1[:], expT[:, 0, :], expT[:, 1, :], AL.add)
                es2 = spool.tile([P, S], BF16, name="es2", tag="es", bufs=3)
                nc.vector.tensor_tensor(es2[:], expT[:, 2, :], expT[:, 3, :], AL.add)
                es = spool.tile([P, S], BF16, name="es", tag="es", bufs=3)
                nc.vector.tensor_tensor(es[:], es1[:], es2[:], AL.add)
                sbc = spool.tile([P, S], BF16, name="sbc", tag="es", bufs=3)
                nc.gpsimd.partition_all_reduce(sbc[:], es[:], P, ReduceOp.add)
                rcb = spool.tile([P, S], BF16, name="rcb", tag="rcb", bufs=2)
                with nc.allow_low_precision(reason="softmax denom recip in bf16"):
                    nc.vector.reciprocal(rcb[:], sbc[:])
                for jc in range(SC):
                    nc.tensor.matmul(
                        ps_c[p0:p0 + DH, :S],
                        lhsT=v_tok[:, jc, h * DH:(h + 1) * DH],
                        rhs=expT[:, jc, :],
                        start=(jc == 0), stop=(jc == SC - 1),
                    )
                nc.vector.tensor_tensor(
                    ctxT[p0:p0 + DH, g, :], ps_c[p0:p0 + DH, :S],
                    rcb[p0:p0 + DH, :], AL.mult,
                )

    # ---- attention out projection + residual + LN1 (stats interleaved) --
    wo = wpool.tile([P, KF, H], BF16, name="wo", tag="w", bufs=3)
    nc.sync.dma_start(out=wo[:], in_=d["aw"][l, 3])
    bo = bcol(d["ab"][l, 3], "bo")
    _proj_residual_ln(
        nc, c, t, l,
        n_in=KF, lhsT_of=lambda m, kf: wo[:, kf, m * P:(m + 1) * P],
        rhs_of=lambda kf: ctxT[:, kf, :],
        bias_col=bo, res_g_dram=d["gprev"][l], name="ln1",
    )

    with nc.named_scope("ffn"):
        b1 = bpool.tile([P, KFF], F32, name="b1", tag="b1col", bufs=2)
        nc.sync.dma_start(out=b1[:], in_=d["b1"][l].rearrange("(ko p) -> p ko", p=P))
        w2tiles = {}

        def w2_prefetch(m):
            if m < KF and m not in w2tiles:
                w2t = wpool.tile([P, KFF, P], BF16, name="w2m", tag="w2", bufs=2)
                nc.sync.dma_start(out=w2t[:], in_=d["w2"][l, m])
                w2tiles[m] = w2t

        w2_prefetch(0)
        for blk in range(FF // 512):
            w1s = wpool.tile([P, KF, 512], BF16, name="w1s", tag="w1", bufs=2)
            nc.sync.dma_start(out=w1s[:], in_=d["w1"][l, blk])
            if blk == 0:
                # kf-outer so the first matmuls only need hT_bf[kf=0]
                psu = [
                    psum.tile([P, 512], F32, name=f"ps_u{j}", tag="pp", bufs=4)
                    for j in range(4)
                ]
                for kf in range(KF):
                    for j in range(4):
                        nc.tensor.matmul(
                            psu[j][:, :S],
                            lhsT=w1s[:, kf, j * P:(j + 1) * P], rhs=hT_bf[:, kf, :],
                            start=(kf == 0), stop=(kf == KF - 1),
                        )
                for j in range(4):
                    nc.scalar.activation(
                        ublk[:, j, :], psu[j][:, :S], AF.Gelu, bias=b1[:, j:j + 1]
                    )
                continue
            for j in range(512 // P):
                kff = blk * 4 + j
                ps_u = psum.tile([P, 512], F32, name="ps_u", tag="pp", bufs=4)
                for kf in range(KF):
                    nc.tensor.matmul(
                        ps_u[:, :S],
                        lhsT=w1s[:, kf, j * P:(j + 1) * P], rhs=hT_bf[:, kf, :],
                        start=(kf == 0), stop=(kf == KF - 1),
                    )
                nc.scalar.activation(
                    ublk[:, kff, :], ps_u[:, :S], AF.Gelu, bias=b1[:, kff:kff + 1]
                )
        b2 = bcol(d["b2"][l], "b2")

        def w2_lhsT(m, kff):
            if kff == 0:
                w2_prefetch(m + 1)
            return w2tiles[m][:, kff, :]

        _proj_residual_ln(
            nc, c, t, l,
            n_in=KFF, lhsT_of=w2_lhsT,
            rhs_of=lambda kff: ublk[:, kff, :],
            bias_col=b2, res_g_dram=d["l1g"][l], name="ln2",
        )


def _proj_residual_ln(nc, c, t, l, n_in, lhsT_of, rhs_of, bias_col,
                      res_g_dram, name):
    """r[m] = xhat_prev[m]*res_g[m] + (sum_k lhsT(m,k).T @ rhs(k) + bias);
    LayerNorm stats of r accumulate on the fly; then hT <- xhat(r) in place
    (gamma/beta of this LN are folded into downstream weights on the host)
    and hT_bf is refreshed."""
    S, H, KF = c["S"], c["H"], c["KF"]
    AL = mybir.AluOpType
    AF = mybir.ActivationFunctionType
    hT, hT_bf = t["hT"], t["hT_bf"]
    ones_col_bf, ones_row = t["ones_col_bf"], t["ones_row"]
    ones_row_f, ident_bf = t["ones_row_f"], t["ident_bf"]
    eps_col = t["eps_col"]
    spool, bpool, psum = t["spool"], t["bpool"], t["psum"]

    with nc.named_scope(name):
        # preload the sqrt act-table during the projection matmuls so the
        # LN chain's Sqrt doesn't wait on a 1.3us table load
        dmy = spool.tile([1, 1], F32, name=f"{name}_dmy", tag="dummy", bufs=2)
        nc.scalar.activation(dmy[:], eps_col[:1, :1], AF.Sqrt)
        resg = bpool.tile([P, KF], F32, name=f"{name}_rg", tag="bcol", bufs=4)
        nc.sync.dma_start(out=resg[:], in_=res_g_dram.rearrange("(ko p) -> p ko", p=P))

        s1_ps = psum.tile([1, 512], F32, name=f"{name}_s1", tag="pst", bufs=2)
        s2_ps = psum.tile([1, 512], F32, name=f"{name}_s2", tag="pst", bufs=2)

        for m in range(KF):
            ps = psum.tile([P, 512], F32, name=f"{name}_po", tag="pc", bufs=2)
            for k in range(n_in):
                nc.tensor.matmul(
                    ps[:, :S], lhsT=lhsT_of(m, k), rhs=rhs_of(k),
                    start=(k == 0), stop=(k == n_in - 1),
                )
            a_sb = spool.tile([P, S], F32, name=f"{name}_a", tag="row_s", bufs=3)
            nc.scalar.activation(a_sb[:], ps[:, :S], AF.Identity,
                                 bias=bias_col[:, m:m + 1])
            # r = xhat_prev * res_g + (proj + folded biases)
            nc.vector.scalar_tensor_tensor(
                out=hT[:, m, :], in0=hT[:, m, :], scalar=resg[:, m:m + 1],
                in1=a_sb[:], op0=AL.mult, op1=AL.add,
            )
            tb = spool.tile([P, S], BF16, name=f"{name}_tb", tag="tb", bufs=2)
            nc.scalar.activation(tb[:], hT[:, m, :], AF.Identity)
            nc.tensor.matmul(
                s1_ps[:1, :S], lhsT=ones_col_bf[:, :1], rhs=tb[:],
                start=(m == 0), stop=(m == KF - 1),
            )
            sq = spool.tile([P, S], BF16, name=f"{name}_sq", tag="sq", bufs=2)
            nc.scalar.activation(sq[:], tb[:], AF.Square)
            nc.tensor.matmul(
                s2_ps[:1, :S], lhsT=ones_col_bf[:, :1], rhs=sq[:],
                start=(m == 0), stop=(m == KF - 1),
            )

        # baseline-form scalar chain on [1, S] rows
        mu = spool.tile([1, S], F32, name=f"{name}_mu", tag="lrow", bufs=3)
        nc.vector.tensor_scalar(mu[:], s1_ps[:1, :S], 1.0 / H, None, AL.mult)
        e2 = spool.tile([1, S], F32, name=f"{name}_e2", tag="lrow", bufs=3)
        nc.vector.tensor_scalar(e2[:], s2_ps[:1, :S], 1.0 / H, None, AL.mult)
        var = spool.tile([1, S], F32, name=f"{name}_var", tag="lrow", bufs=3)
        nc.vector.tensor_tensor(var[:], mu[:], mu[:], AL.mult)
        nc.vector.tensor_tensor(var[:], e2[:], var[:], AL.subtract)
        sd = spool.tile([1, S], F32, name=f"{name}_sd", tag="lrow", bufs=3)
        nc.scalar.activation(sd[:], var[:], AF.Sqrt, bias=eps_col[:1, :1])
        rstd = spool.tile([1, S], F32, name=f"{name}_rstd", tag="lrow", bufs=3)
        nc.vector.reciprocal(rstd[:], sd[:])
        mrs = spool.tile([1, S], F32, name=f"{name}_mrs", tag="lrow", bufs=3)
        nc.vector.tensor_tensor(mrs[:], mu[:], rstd[:], AL.mult)
        nc.vector.tensor_scalar(mrs[:], mrs[:], -1.0, None, AL.mult)

        # broadcast rstd / -mu*rstd across partitions (fp32 matmuls, proven)
        rb_ps = psum.tile([P, 512], F32, name=f"{name}_rb", tag="pst", bufs=2)
        nc.tensor.matmul(rb_ps[:, :S], lhsT=ones_row_f[:], rhs=rstd[:],
                         start=True, stop=True)
        mb_ps = psum.tile([P, 512], F32, name=f"{name}_mb", tag="pst", bufs=2)
        nc.tensor.matmul(mb_ps[:, :S], lhsT=ones_row_f[:], rhs=mrs[:],
                         start=True, stop=True)

        dmy2 = spool.tile([1, 1], F32, name=f"{name}_dmy2", tag="dummy", bufs=2)
        nc.scalar.activation(dmy2[:], eps_col[:1, :1],
                             AF.Gelu if name == "ln1" else AF.Exp)
        for kf in range(KF):
            tt = spool.tile([P, S], F32, name=f"{name}_t", tag="row_s", bufs=3)
            nc.vector.tensor_tensor(tt[:], rb_ps[:, :S], hT[:, kf, :], AL.mult)
            nc.vector.tensor_tensor(hT[:, kf, :], mb_ps[:, :S], tt[:], AL.add)
            nc.scalar.activation(hT_bf[:, kf, :], hT[:, kf, :], AF.Identity)


# =========================================================================
# Host entry point
# =========================================================================

_NC_CACHE = {}


def _get_nc():
    key = "full"
    if key not in _NC_CACHE:
        _NC_CACHE[key] = build_nc(FULL_CFG)
    return _NC_CACHE[key]


def _prep_in_maps(inputs):
    import ml_dtypes

    cfg = FULL_CFG
    B = N_CORES
    L, H, FF, V, S = cfg["L"], cfg["H"], cfg["FF"], cfg["V"], cfg["S"]
    ids = np.asarray(inputs["input_ids"], dtype=np.int32)
    assert ids.shape == (B, S)

    pos_type = (
        np.asarray(inputs["pos_emb"], np.float32)
        + np.asarray(inputs["type_emb"], np.float32)[0][None, :]
    )
    bf = lambda x: np.ascontiguousarray(
        np.asarray(x, np.float32).astype(ml_dtypes.bfloat16)
    )

    # ---- LayerNorm gamma/beta folding into downstream weights ----------
    # hT stores xhat (un-affine LN output); each consumer's weights absorb
    # the producing LN's gamma (rows scaled) and beta (folded into biases);
    # the residual adds xhat*gprev + (proj + bias + bprev).
    emb_g = np.asarray(inputs["emb_ln_g"], np.float32)
    emb_b = np.asarray(inputs["emb_ln_b"], np.float32)
    l1g = np.asarray(inputs["ln1_g"], np.float32)
    l1b = np.asarray(inputs["ln1_b"], np.float32)
    l2g = np.asarray(inputs["ln2_g"], np.float32)
    l2b = np.asarray(inputs["ln2_b"], np.float32)
    gprev = np.concatenate([emb_g[None], l2g[:-1]], axis=0)   # [L, H]
    bprev = np.concatenate([emb_b[None], l2b[:-1]], axis=0)   # [L, H]

    aw = np.asarray(inputs["attn_w"], np.float32).copy()      # [L, 4, H, H]
    ab = np.asarray(inputs["attn_b"], np.float32).copy()      # [L, 4, H]
    for i in range(3):  # q, k, v read xhat_prev
        ab[:, i] += np.einsum("lh,lhf->lf", bprev, aw[:, i])
        aw[:, i] *= gprev[:, :, None]
    ab[:, 3] += bprev  # residual's +bprev rides the o-proj bias
    aw_x = np.ascontiguousarray(
        aw.reshape(L, 4, H // P, P, H).transpose(0, 1, 3, 2, 4)
    )                                                         # [L,4,P,KF,H]

    w1 = np.asarray(inputs["ffn_w1"], np.float32).copy()      # [L, H, FF]
    b1 = np.asarray(inputs["ffn_b1"], np.float32).copy()
    b1 += np.einsum("lh,lhf->lf", l1b, w1)
    w1 *= l1g[:, :, None]
    w1_x = np.ascontiguousarray(
        w1.reshape(L, H // P, P, FF // 512, 512).transpose(0, 3, 2, 1, 4)
    )                                                         # [L,6,P,KF,512]

    w2 = np.asarray(inputs["ffn_w2"], np.float32)             # [L, FF, H]
    b2 = np.asarray(inputs["ffn_b2"], np.float32) + l1b       # residual +l1b
    w2_x = np.ascontiguousarray(
        w2.reshape(L, FF // P, P, H // P, P).transpose(0, 3, 2, 1, 4)
    )                                                         # [L,6,P,24,128]

    cls = np.zeros((H, V_PAD), np.float32)
    cls[:, :V] = np.asarray(inputs["cls_w"], np.float32)
    clsb = np.zeros((V_PAD,), np.float32)
    clsb[:V] = np.asarray(inputs["cls_b"], np.float32)
    clsb[:V] += l2b[-1] @ cls[:, :V]
    cls *= l2g[-1][:, None]
    cls_x = np.ascontiguousarray(
        cls.reshape(H // P, P, V_PAD // 512, 512).transpose(2, 1, 0, 3)
    )                                                         # [42,P,KF,512]

    shared = {
        "word_emb": np.ascontiguousarray(inputs["word_emb"], np.float32),
        "pos_type_emb": np.ascontiguousarray(pos_type, np.float32),
        "attn_wx": bf(aw_x),
        "attn_b": np.ascontiguousarray(ab),
        "ln1_g": np.ascontiguousarray(l1g),
        "gprev": np.ascontiguousarray(gprev),
        "ffn_w1x": bf(w1_x),
        "ffn_b1": np.ascontiguousarray(b1),
        "ffn_w2x": bf(w2_x),
        "ffn_b2": np.ascontiguousarray(b2),
        "cls_wx": bf(cls_x),
        "cls_b": bf(clsb),
    }
    in_maps = [
        {"input_ids": np.ascontiguousarray(ids[i]), **shared} for i in range(B)
    ]
    return in_maps


def _run(inputs, trace=False, **kw):
    from concourse.bass_utils import run_bass_kernel_spmd

    in_maps = _prep_in_maps(inputs)
    nc = _get_nc()
    res = run_bass_kernel_spmd(nc, in_maps, list(range(N_CORES)), trace=trace, **kw)
    out = np.stack(
        [res.results[i]["out"] for i in range(N_CORES)], axis=0
    ).astype(np.float32)
    return out, res


def kernel(**inputs):
    out, _ = _run(inputs, trace=False)
    return out


def run_traced(**inputs):
    return _run(inputs, trace=True)
